# revision 46
# baseline (speedup 1.0000x reference)
"""GraphSAGE (mean aggregation) on 8 Trainium2 NeuronCores.

Strategy (v8):
  - Nodes partitioned across 8 cores (6250 real + pad -> 6400/core).
  - Full node-feature table replicated in each core's DRAM as NCH chunk
    tensors [NC*RCH, 128] f16 (chunk q holds rows q*RCH..(q+1)*RCH-1 of every
    core, m-major). Each chunk is refreshed by its own AllGather issued as
    soon as its WCH windows are written back -> all but the last collective
    hide under compute.
  - Edge messages fetched with dma_gather (custom SWDGE gather, 4 queues,
    int16 indices into the chunk tables). fp16 rows (256B) feed PE directly.
  - Mean aggregation = PE matmuls: per 128-edge block, lhsT = gathered
    messages [128e, 128h] (fp16), rhs = selection matrix S [128e, 128] with
    1/deg folded in. S blocks are static graph structure: precomputed on the
    host, uploaded once to DRAM, streamed per-window into SBUF in one DMA
    (no per-block DVE work at all).
  - Dense SAGE transform per window: zT = Wl^T aggT + Wr^T hT + b; bias+relu
    and PSUM->SBUF copies on the Activation engine.
  - h' transposed back to node-major via PE and written to cc_in chunks.
"""
import sys

sys.path.insert(0, "/opt/trn_rl_repo")

import numpy as np

import concourse.bass as bass
import concourse.bacc as bacc
import concourse.tile as tile
from concourse import mybir, library_config
from concourse.masks import make_identity

# problem constants (hardcoded per contract)
N, E, IN_DIM, HID, L = 50000, 625000, 300, 128, 4
NC = 8
NPC = N // NC            # 6250 real nodes per core
W_N = 128                # aggregation window width (psum free dim)
NW = 50                  # windows per core
NPAD = W_N * NW          # 6400 padded nodes per core
NTAB = NC * NPAD         # 51200 rows in the replicated table
KCH = 3                  # 384 = 3*128 >= IN_DIM contraction chunks
import os as _os_mod
GMAX = int(_os_mod.environ.get("KERNEL_GMAX", "1024"))  # indices per dma_gather
NCH = int(_os_mod.environ.get("KERNEL_AG_CHUNKS", "2"))  # table/collective chunks
WCH = NW // NCH          # windows per table chunk
RCH = WCH * W_N          # rows per table chunk per core
TROWS = NC * RCH         # rows per chunk table (m-major); must fit int16
assert NW % NCH == 0 and TROWS <= 32767

_CACHE = {}


def _host_prep(edge_index):
    """Build per-core gather streams, S-block metadata and program structure."""
    src = edge_index[0].astype(np.int64)
    dst = edge_index[1].astype(np.int64)
    # padded global ids
    gsrc = (src // NPC) * NPAD + (src % NPC)
    gdst = (dst // NPC) * NPAD + (dst % NPC)
    # source chunk table + row within it (m-major chunk layout)
    sseg_all = (gsrc % NPAD) // RCH
    stok_all = (gsrc // NPAD) * RCH + (gsrc % NPAD) % RCH

    per_core = []
    counts = np.zeros((NC, NCH, NW), dtype=np.int64)
    for m in range(NC):
        sel = (gdst // NPAD) == m
        s_m = stok_all[sel]
        seg = sseg_all[sel]
        dl = (gdst[sel] - m * NPAD).astype(np.int64)   # 0..6249
        w = dl // W_N
        # sort by (seg, dl) stable
        order = np.lexsort((dl, seg))
        s_m, dl, seg, w = s_m[order], dl[order], seg[order], w[order]
        per_core.append((s_m, dl, seg, w))
        for h in range(NCH):
            cw = np.bincount(w[seg == h], minlength=NW)
            counts[m, h, :] = cw

    # uniform block structure across cores
    B = np.zeros((NCH, NW), dtype=np.int64)
    for h in range(NCH):
        for w in range(NW):
            B[h, w] = int(np.ceil(counts[:, h, w].max() / 128.0))

    # stream slot layout: per seg, concat over windows of B[h,w]*128 slots
    slots_h = [int(B[h].sum() * 128) for h in range(NCH)]

    # per-(h,w) slot offsets
    slot_off = np.zeros((NCH, NW), dtype=np.int64)
    for h in range(NCH):
        acc = 0
        for w in range(NW):
            slot_off[h, w] = acc
            acc += B[h, w] * 128

    # S-stream block order: for w: for h: for j in B[h,w]. sb_off[w] = first
    # block of window w in the S stream.
    sb_off = np.zeros(NW + 1, dtype=np.int64)
    for w in range(NW):
        sb_off[w + 1] = sb_off[w] + int(B[:, w].sum())
    totb = int(sb_off[NW])

    # gather calls: per seg, chunks split at (h,w) boundaries ("window" mode;
    # "merge" mode measured slower on HW).
    chunk_mode = _os_mod.environ.get("KERNEL_CHUNK_MODE", "window")
    chunk_list = [[] for _ in range(NCH)]  # per seg: [(start_slot, n), ...]
    if chunk_mode == "merge":
        slots_pad = [
            int(np.ceil(max(slots_h[h], 1) / GMAX) * GMAX) for h in range(NCH)
        ]
        for h in range(NCH):
            for w0 in range(0, slots_pad[h], GMAX):
                chunk_list[h].append((w0, GMAX))
    else:
        slots_pad = [max(slots_h[h], 1) for h in range(NCH)]
        for h in range(NCH):
            for w in range(NW):
                nslots = int(B[h, w] * 128)
                off = int(slot_off[h, w])
                while nslots > 0:
                    n = min(GMAX, nslots)
                    chunk_list[h].append((off, n))
                    off += n
                    nslots -= n
    # block -> chunk index map per seg
    blk2chunk = []
    for h in range(NCH):
        m = np.zeros(max(slots_h[h] // 128, 1), dtype=np.int64)
        for ci, (w0, n) in enumerate(chunk_list[h]):
            for b in range(w0 // 128, min((w0 + n) // 128, len(m))):
                m[b] = ci
        blk2chunk.append(m)

    idx_wrapped = []   # per core: [NCH][128, slots_pad/16] int16
    dof_core = []      # per core: [NCH] dst-offset arrays (slot-indexed), -1 pad
    nval_core = []     # per core: [NCH] per-call valid-index counts (int32)
    for m in range(NC):
        s_m, dl, seg, w = per_core[m]
        iw_list, dof_list, nv_list = [], [], []
        for h in range(NCH):
            # pad tokens are -1: SWDGE skips trailing negatives (no
            # descriptor, no transfer); S rows for those slots are zero.
            tok = np.full(slots_pad[h], -1, dtype=np.int16)
            dof = np.full(slots_h[h], -1, dtype=np.int64)
            sel = seg == h
            s_h, dl_h, w_h = s_m[sel], dl[sel], w[sel]
            for wi in range(NW):
                selw = w_h == wi
                cnt = int(selw.sum())
                if cnt == 0:
                    continue
                o = int(slot_off[h, wi])
                tok[o : o + cnt] = s_h[selw].astype(np.int16)
                dof[o : o + cnt] = dl_h[selw] - wi * W_N
            # per-call valid counts; guarantee >= 1 valid index per call
            nv = np.zeros(len(chunk_list[h]), dtype=np.int32)
            for ci, (w0, n) in enumerate(chunk_list[h]):
                cnt = int((tok[w0 : w0 + n] >= 0).sum())
                if cnt == 0:
                    tok[w0] = np.int16((w0 * 257) % TROWS)
                    cnt = 1
                nv[ci] = cnt
            # wrap idx per gather call: j -> [j%16, j//16], replicated x8
            iw = np.zeros((128, slots_pad[h] // 16), dtype=np.int16)
            for (w0, n) in chunk_list[h]:
                blockw = tok[w0 : w0 + n].reshape(n // 16, 16).T
                iw[:, w0 // 16 : (w0 + n) // 16] = np.tile(blockw, (8, 1))
            iw_list.append(iw)
            dof_list.append(dof)
            nv_list.append(nv)
        idx_wrapped.append(iw_list)
        dof_core.append(dof_list)
        nval_core.append(nv_list)

    return {
        "nval": nval_core,
        "B": B,
        "slots_h": slots_h,
        "slots_pad": slots_pad,
        "chunk_list": chunk_list,
        "blk2chunk": blk2chunk,
        "slot_off": slot_off,
        "sb_off": sb_off,
        "totb": totb,
        "idx_wrapped": idx_wrapped,
        "dof": dof_core,
    }


def _build_program(struct, timing_reps=1):
    B = struct["B"]
    slots_pad = struct["slots_pad"]
    chunk_list = struct["chunk_list"]
    blk2chunk = struct["blk2chunk"]
    sb_off = struct["sb_off"]
    totb = struct["totb"]

    nc = bacc.Bacc(
        "TRN2",
        target_bir_lowering=False,
        debug=False,
        num_devices=NC,
        num_swdge_queues=4,
    )
    f32, f16, i16 = mybir.dt.float32, mybir.dt.float16, mybir.dt.int16

    idx_d = [
        nc.dram_tensor(f"idx{h}", [128, max(slots_pad[h] // 16, 1)], i16, kind="ExternalInput")
        for h in range(NCH)
    ]
    i32 = mybir.dt.int32
    nv_d = [
        nc.dram_tensor(f"nv{h}", [1, max(len(chunk_list[h]), 1)], i32, kind="ExternalInput")
        for h in range(NCH)
    ]
    S_d = nc.dram_tensor("Sb", [128, totb, 128], f16, kind="ExternalInput")
    xT_d = nc.dram_tensor("xT", [KCH, 128, NPAD], f16, kind="ExternalInput")
    embW_d = nc.dram_tensor("embW", [KCH, 128, HID], f16, kind="ExternalInput")
    embB_d = nc.dram_tensor("embB", [128, 1], f32, kind="ExternalInput")
    Wl_d = nc.dram_tensor("Wl", [L, 128, HID], f16, kind="ExternalInput")
    Wr_d = nc.dram_tensor("Wr", [L, 128, HID], f16, kind="ExternalInput")
    bl_d = nc.dram_tensor("bl", [L, 128, 1], f32, kind="ExternalInput")
    out_d = nc.dram_tensor("out", [NPAD, HID], f32, kind="ExternalOutput")

    rg = [list(range(NC))]
    qctr = [0]

    def next_q():
        q = qctr[0] % 4
        qctr[0] += 1
        return q

    import os as _os
    _trace = _os.environ.get("KERNEL_TRACE_SIM") == "1"
    _abl = _os.environ.get("KERNEL_ABLATE", "")
    _ablate = _abl == "1"
    _no_ag = _os.environ.get("KERNEL_NO_AG") == "1"
    with tile.TileContext(nc, trace_sim=_trace) as tc:
        GTB = int(_os.environ.get("KERNEL_GT_BUFS", "16"))
        PAGB = int(_os.environ.get("KERNEL_PAG_BUFS", "4"))
        SWB = int(_os.environ.get("KERNEL_SW_BUFS", "4"))
        with (
            tc.tile_pool(name="const", bufs=1) as constp,
            tc.tile_pool(name="big", bufs=1) as bigp,
            tc.tile_pool(name="gt", bufs=GTB) as gtp,
            tc.tile_pool(name="sw", bufs=SWB) as swp,
            tc.tile_pool(name="ap", bufs=4) as apool,
            tc.tile_pool(name="hp", bufs=4) as hpool,
            tc.tile_pool(name="pag", bufs=PAGB, space="PSUM") as pag,
            tc.tile_pool(name="pz", bufs=2, space="PSUM") as pz,
            tc.tile_pool(name="pt", bufs=2, space="PSUM") as pt,
            tc.tile_pool(name="dram", bufs=1, space="DRAM") as dram,
        ):
            nc.gpsimd.load_library(library_config.mlp)

            # --- resident constants / inputs in SBUF ---
            idx_sb = []
            nv_sb = []
            for h in range(NCH):
                t = constp.tile([128, max(slots_pad[h] // 16, 1)], i16, name=f"idxsb{h}")
                nc.sync.dma_start(out=t[:], in_=idx_d[h][:])
                idx_sb.append(t)
                tn = constp.tile([1, max(len(chunk_list[h]), 1)], i32, name=f"nvsb{h}")
                nc.sync.dma_start(out=tn[:], in_=nv_d[h][:])
                nv_sb.append(tn)
            nreg = nc.gpsimd.alloc_register("nval")
            # init gather-pool slots once: rows skipped by negative indices
            # keep stale SBUF bytes, which must not be NaN (S*NaN poisons psum)
            for _ in range(GTB):
                gi = gtp.tile([128, GMAX // 128, 128], f16, tag="gt", name="gt")
                nc.vector.memset(gi[:], 0.0)
            ident = constp.tile([128, 128], f32)
            make_identity(nc, ident[:])
            ident16 = constp.tile([128, 128], f16)
            nc.vector.tensor_copy(ident16[:], ident[:])
            embW_sb = constp.tile([128, KCH, HID], f16)
            nc.sync.dma_start(out=embW_sb[:], in_=embW_d[:].rearrange("k p h -> p k h"))
            embB_sb = constp.tile([128, 1], f32)
            nc.sync.dma_start(out=embB_sb[:], in_=embB_d[:])
            Wl_sb = constp.tile([128, L, HID], f16)
            nc.sync.dma_start(out=Wl_sb[:], in_=Wl_d[:].rearrange("l p h -> p l h"))
            Wr_sb = constp.tile([128, L, HID], f16)
            nc.sync.dma_start(out=Wr_sb[:], in_=Wr_d[:].rearrange("l p h -> p l h"))
            bl_sb = constp.tile([128, L], f32)
            nc.sync.dma_start(out=bl_sb[:], in_=bl_d[:].rearrange("l p one -> p (l one)"))
            xT_sb = bigp.tile([128, KCH, NPAD], f16)
            nc.sync.dma_start(out=xT_sb[:], in_=xT_d[:].rearrange("k p n -> p k n"))

            hT = [bigp.tile([128, NPAD], f16, name=f"hT{i}") for i in range(2)]

            # DRAM buffers; cc_in split per collective chunk so each chunk's
            # AllGather depends only on its own windows' writebacks. The
            # feature table is NCH Shared chunk tensors (single-writer rule).
            n_ag = 1 + timing_reps * (L - 1)
            cc_in = [
                [
                    dram.tile([RCH, HID], f16, name=f"ccin{i}_{q}", bufs=1)
                    for q in range(NCH)
                ]
                for i in range(2)
            ]
            h_all = [
                [
                    dram.tile(
                        [TROWS, HID], f16, name=f"hall{i}_{q}", bufs=1,
                        addr_space="Shared",
                    )
                    for q in range(NCH)
                ]
                for i in range(n_ag)
            ]

            def embedding():
                for w in range(NW):
                    ws = slice(w * W_N, (w + 1) * W_N)
                    pzz = pz.tile([128, W_N], f32, tag="pz", name="pz")
                    for k in range(KCH):
                        nc.tensor.matmul(
                            pzz[:],
                            lhsT=embW_sb[:, k, :],
                            rhs=xT_sb[:, k, ws],
                            start=(k == 0),
                            stop=(k == KCH - 1),
                        )
                    nc.scalar.activation(
                        hT[0][:, ws], pzz[:], mybir.ActivationFunctionType.Relu,
                        bias=embB_sb[:],
                    )
                    writeback(hT[0], w, cc_in[0])
                    if (w + 1) % WCH == 0:
                        allgather_chunk(cc_in[0], h_all[0], w // WCH)

            def writeback(hTbuf, w, dest_list):
                # transpose window back to node-major and DMA to chunk rows
                cs = slice(w * W_N, (w + 1) * W_N)
                q, wq = divmod(w, WCH)
                ptile = pt.tile([128, 128], f16, tag="pt16", name="ptile")
                nc.tensor.transpose(ptile[:], hTbuf[:, cs], ident16[:])
                hsb = hpool.tile([128, 128], f16, tag="hsb", name="hsb")
                nc.scalar.copy(hsb[:], ptile[:])
                nc.sync.dma_start(
                    out=dest_list[q][wq * W_N : (wq + 1) * W_N, :], in_=hsb[:]
                )

            def allgather_chunk(src_cc_list, dst_halls, q):
                nc.gpsimd.collective_compute(
                    "AllGather",
                    mybir.AluOpType.bypass,
                    replica_groups=rg,
                    ins=[src_cc_list[q][:]],
                    outs=[dst_halls[q][:]],
                )

            AGD = int(_os.environ.get("KERNEL_AG_DELAY", "6"))
            # chunk q's AllGather is emitted after window (q+1)*WCH-1+AGD so
            # Pool (running ahead on gather desc-gen) doesn't stall on the
            # writeback wait; the last chunk goes at the final window.
            ag_at = {}
            for q in range(NCH):
                wq = min(NW - 1, (q + 1) * WCH - 1 + AGD)
                ag_at.setdefault(wq, []).append(q)

            def layer(l, h_src, hT_in, hT_out, dest, ag_dest=None):
                half_ap = [h_src[q][:] for q in range(NCH)]
                issued = [0] * NCH     # gather calls issued so far, per seg
                chunk_tiles = [{} for _ in range(NCH)]

                def ensure_chunk(h, ci):
                    while issued[h] <= ci:
                        c = issued[h]
                        w0, n = chunk_list[h][c]
                        gt = gtp.tile(
                            [128, n // 128, 128], f16, tag="gt", name="gt"
                        )
                        nc.gpsimd.reg_load(nreg, nv_sb[h][0:1, c : c + 1])
                        nc.gpsimd.dma_gather(
                            gt[:],
                            half_ap[h],
                            idx_sb[h][:, w0 // 16 : (w0 + n) // 16],
                            n,
                            nreg,
                            HID,
                            queue_num=next_q(),
                        )
                        chunk_tiles[h][c] = (gt, w0)
                        if c - GTB in chunk_tiles[h]:
                            del chunk_tiles[h][c - GTB]
                        issued[h] += 1

                if _ablate:
                    for h in range(NCH):
                        ensure_chunk(h, len(chunk_list[h]) - 1)

                gb0 = [0] * NCH        # global block offset per seg
                for w in range(NW):
                    pagg = pag.tile([128, W_N], f32, tag="pagg", name="pagg")
                    nblocks = int(B[:, w].sum())
                    # stream this window's S blocks from DRAM in one DMA
                    if nblocks > 0 and not _ablate:
                        Sw = swp.tile([128, nblocks, 128], f16, tag="Sw", name="Sw")
                        nc.sync.dma_start(
                            out=Sw[:],
                            in_=S_d[:, sb_off[w] : sb_off[w + 1], :],
                        )
                    first = True
                    done = 0
                    sblk = 0
                    for h in range(NCH):
                        for j in range(int(B[h][w])):
                            gb = gb0[h] + j
                            if _ablate:
                                done += 1
                                continue
                            ci = int(blk2chunk[h][gb])
                            ensure_chunk(h, ci)
                            gt, cw0 = chunk_tiles[h][ci]
                            done += 1
                            nc.tensor.matmul(
                                pagg[:],
                                lhsT=gt[:, (gb * 128 - cw0) // 128, :],
                                rhs=Sw[:, sblk + j, :],
                                start=first,
                                stop=(done == nblocks),
                            )
                            first = False
                        gb0[h] += int(B[h][w])
                        sblk += int(B[h][w])
                    ws = slice(w * W_N, (w + 1) * W_N)
                    if _ablate:
                        first = True
                    aggT = apool.tile([128, W_N], f16, tag="aggT", name="aggT")
                    if first:
                        nc.vector.memset(aggT[:], 0.0)
                    else:
                        nc.scalar.copy(aggT[:], pagg[:])
                    pzz = pz.tile([128, W_N], f32, tag="pz", name="pz")
                    nc.tensor.matmul(
                        pzz[:], lhsT=Wl_sb[:, l, :], rhs=aggT[:], start=True, stop=False
                    )
                    nc.tensor.matmul(
                        pzz[:], lhsT=Wr_sb[:, l, :], rhs=hT_in[:, ws], start=False,
                        stop=True,
                    )
                    if l < L - 1:
                        nc.scalar.activation(
                            hT_out[:, ws], pzz[:], mybir.ActivationFunctionType.Relu,
                            bias=bl_sb[:, l : l + 1],
                        )
                        writeback(hT_out, w, dest)
                        if ag_dest is not None and w in ag_at:
                            for q in ag_at[w]:
                                allgather_chunk(dest, ag_dest, q)
                    else:
                        h4 = apool.tile([128, W_N], f16, tag="h4", name="h4")
                        nc.scalar.activation(
                            h4[:], pzz[:], mybir.ActivationFunctionType.Relu,
                            bias=bl_sb[:, l : l + 1],
                        )
                        ptile = pt.tile([128, 128], f16, tag="pt16", name="ptile")
                        nc.tensor.transpose(ptile[:], h4[:], ident16[:])
                        hsb = hpool.tile([128, 128], f32, tag="hsbo", name="hsbo")
                        nc.vector.tensor_copy(hsb[:], ptile[:])
                        nc.sync.dma_start(out=out_d[ws, :], in_=hsb[:])

            embedding()
            agi = 0
            for rep in range(timing_reps):
                for l in range(L):
                    src_idx = agi
                    if l < L - 1 and not _no_ag:
                        agi += 1
                        ag_dest = h_all[agi]
                    else:
                        ag_dest = None
                    layer(
                        l,
                        h_all[src_idx],
                        hT[l % 2],
                        hT[(l + 1) % 2],
                        cc_in[(l + 1) % 2],
                        ag_dest,
                    )

    nc.compile()
    return nc


def _prep_inputs(inputs, struct):
    x = np.asarray(inputs["x"], dtype=np.float32)
    emb_W = np.asarray(inputs["emb_W"], dtype=np.float32)
    emb_b = np.asarray(inputs["emb_b"], dtype=np.float32)
    Wl = np.asarray(inputs["Wl"], dtype=np.float32)
    bl = np.asarray(inputs["bl"], dtype=np.float32)
    Wr = np.asarray(inputs["Wr"], dtype=np.float32)
    edge_index = np.asarray(inputs["edge_index"])

    embW_p = np.zeros((KCH, 128, HID), dtype=np.float16)
    embW_p.reshape(KCH * 128, HID)[:IN_DIM] = emb_W.astype(np.float16)
    embB_p = np.zeros((128, 1), dtype=np.float32)
    embB_p[:, 0] = emb_b
    Wl_p = Wl.astype(np.float16)
    Wr_p = Wr.astype(np.float16)
    bl_p = np.ascontiguousarray(bl[:, :, None].astype(np.float32))

    # host-precomputed 1/deg (clamped)
    deg = np.bincount(edge_index[1].astype(np.int64), minlength=N).astype(np.float32)
    inv_full = 1.0 / np.maximum(deg, 1.0)

    B = struct["B"]
    sb_off = struct["sb_off"]
    slot_off = struct["slot_off"]
    totb = struct["totb"]

    in_maps = []
    for m in range(NC):
        xm = np.zeros((KCH * 128, NPAD), dtype=np.float16)
        xm[:IN_DIM, :NPC] = x[m * NPC : (m + 1) * NPC].T.astype(np.float16)
        invm = np.ones((NPAD,), dtype=np.float32)
        invm[:NPC] = inv_full[m * NPC : (m + 1) * NPC]
        # Build the S stream, stored edge-partition-major [128e, totb, 128d]
        # f16 so each window's S-load DMA is 128 contiguous multi-KB runs.
        # Block order (w, h, j); S[e, b, d] = invm[w*128+d] if slot e of block
        # (h,w,j) targets local offset d, else 0.
        S = np.zeros((totb, 128, 128), dtype=np.float16)
        dof = struct["dof"][m]
        for w in range(NW):
            b = int(sb_off[w])
            for h in range(NCH):
                nb = int(B[h][w])
                if nb == 0:
                    continue
                o = int(slot_off[h][w])
                d = dof[h][o : o + nb * 128].reshape(nb, 128)  # [-1 or 0..127]
                blk, e = np.nonzero(d >= 0)
                dv = d[blk, e]
                S[b + blk, e, dv] = invm[w * W_N + dv].astype(np.float16)
                b += nb
        S = np.ascontiguousarray(S.transpose(1, 0, 2))
        im = {"Sb": S, "xT": xm.reshape(KCH, 128, NPAD), "embW": embW_p,
              "embB": embB_p, "Wl": Wl_p, "Wr": Wr_p, "bl": bl_p}
        for h in range(NCH):
            im[f"idx{h}"] = struct["idx_wrapped"][m][h]
            im[f"nv{h}"] = struct["nval"][m][h][None, :]
        in_maps.append(im)
    return in_maps


class BassRunner:
    """Executes a compiled Bass program via PJRT/axon; jit built once."""

    def __init__(self, nc, n_cores):
        import jax
        from jax.sharding import Mesh, PartitionSpec
        from jax.experimental.shard_map import shard_map
        from concourse.bass2jax import (
            _bass_exec_p,
            install_neuronx_cc_hook,
            partition_id_tensor,
        )

        install_neuronx_cc_hook()
        self.jax = jax
        self.nc = nc
        self.n_cores = n_cores
        partition_name = (
            nc.partition_id_tensor.name if nc.partition_id_tensor else None
        )
        in_names, out_names, out_avals, zero_outs = [], [], [], []
        for alloc in nc.m.functions[0].allocations:
            if not isinstance(alloc, mybir.MemoryLocationSet):
                continue
            name = alloc.memorylocations[0].name
            if alloc.kind == "ExternalInput":
                if name != partition_name:
                    in_names.append(name)
            elif alloc.kind == "ExternalOutput":
                shape = tuple(alloc.tensor_shape)
                dtype = mybir.dt.np(alloc.dtype)
                out_names.append(name)
                out_avals.append(jax.core.ShapedArray(shape, dtype))
                zero_outs.append(np.zeros(shape, dtype))
        self.in_names, self.out_names = in_names, out_names
        self.zero_outs, self._out_avals = zero_outs, out_avals
        n_params, n_outs = len(in_names), len(out_avals)
        all_in_names = in_names + out_names
        if partition_name is not None:
            all_in_names = all_in_names + [partition_name]

        def _body(*args):
            operands = list(args)
            if partition_name is not None:
                operands.append(partition_id_tensor())
            return tuple(
                _bass_exec_p.bind(
                    *operands,
                    out_avals=tuple(out_avals),
                    in_names=tuple(all_in_names),
                    out_names=tuple(out_names),
                    lowering_input_output_aliases=(),
                    sim_require_finite=True,
                    sim_require_nnan=True,
                    nc=nc,
                )
            )

        devices = jax.devices()[:n_cores]
        self._mesh = Mesh(np.asarray(devices), ("core",))
        self._pspec = PartitionSpec("core")
        in_specs = (self._pspec,) * (n_params + n_outs)
        out_specs = (self._pspec,) * len(out_names)
        self._fn = jax.jit(
            shard_map(
                _body,
                mesh=self._mesh,
                in_specs=in_specs,
                out_specs=out_specs,
                check_rep=False,
            ),
            keep_unused=True,
        )

    def prepare(self, in_maps):
        n = self.n_cores
        concat_in = [
            np.concatenate(
                [np.asarray(in_maps[c][name]) for c in range(n)], axis=0
            )
            for name in self.in_names
        ]
        concat_zeros = [
            np.zeros((n * z.shape[0], *z.shape[1:]), z.dtype)
            for z in self.zero_outs
        ]
        sharding = self.jax.sharding.NamedSharding(self._mesh, self._pspec)
        self._args = [
            self.jax.device_put(a, sharding) for a in concat_in + concat_zeros
        ]

    def execute(self):
        outs = self._fn(*self._args)
        self.jax.block_until_ready(outs)
        return outs

    def run(self):
        outs = self.execute()
        n = self.n_cores
        return [
            {
                name: np.asarray(outs[i]).reshape(
                    n, *self._out_avals[i].shape
                )[c]
                for i, name in enumerate(self.out_names)
            }
            for c in range(n)
        ]


def _get_runner(edge_index, timing_reps=1):
    key = ("prog", timing_reps, hash(edge_index.tobytes()))
    if key in _CACHE:
        return _CACHE[key]
    struct = _host_prep(edge_index)
    nc = _build_program(struct, timing_reps=timing_reps)
    runner = BassRunner(nc, NC)
    _CACHE[key] = (struct, runner)
    return struct, runner


def kernel(**inputs):
    edge_index = np.asarray(inputs["edge_index"])
    struct, runner = _get_runner(edge_index)
    in_maps = _prep_inputs(inputs, struct)
    runner.prepare(in_maps)
    results = runner.run()
    out = np.empty((N, HID), dtype=np.float32)
    for m in range(NC):
        out[m * NPC : (m + 1) * NPC] = results[m]["out"][:NPC]
    return out
